# revision 28
# baseline (speedup 1.0000x reference)
"""ArcFace multi-head-sharded loss on 8 TRN2 NeuronCores.

Strategy: shard the (64, 2048, 256) weight table over the group axis —
each core owns 8 groups. Samples are routed host-side to the core owning
their group (host routing replaces the all-to-all). Weight rows are
l2-normalized host-side and quantized to fp8e4 (x16 pre-scale to stay in
the normal range), so the device only does:

  - stream its 8 weight groups (4MB fp8) from HBM; DMA triggers alternate
    between the two HW-DGE queues (sync + scalar) so descriptor
    generation is not serialized on one engine,
  - mains: cos_raw(b, c) = <xq_b, wq_c> on PE (fp8 x fp8 -> f32 PSUM),
  - exp with fused accumulation over the class axis (scale folds the
    1/256 quantization scale and the ArcFace scale 64),
  - target logit via a per-row dot with the host-gathered target weight
    row (xw . wtar, 256-wide DVE reduce),
  - the margin + CE epilogue on [128,T] vectors (both tiles batched),
  - one partial-loss scalar out (sum of -logp/B over its samples).

Host: sums the 8 scalars. ~4MB HBM traffic per core => memory-bound.

Samples are packed into "bands" of NG=32 partition rows, one band per
weight group (plus overflow bands), 4 bands per 128-row sample tile.
"""

import sys
import numpy as np
import ml_dtypes

BF16 = ml_dtypes.bfloat16
FP8 = ml_dtypes.float8_e4m3

_TRN_REPO = "/opt/trn_rl_repo"
if _TRN_REPO not in sys.path:
    sys.path.insert(0, _TRN_REPO)

# problem config (hardcoded per spec)
B, E, G, C = 512, 256, 64, 2048
NCORES = 8
GPC = G // NCORES        # weight groups per core
NG = 32                  # sample slots per band
BPT = 128 // NG          # bands per 128-partition sample tile
KE = E // 128            # contraction chunks
NCC = C // 512           # 512-col chunks per group
SCALE = 64.0
MARGIN = 0.5
COS_M = float(np.cos(MARGIN))
SIN_M = float(np.sin(MARGIN))
THETA = float(np.cos(np.pi - MARGIN))
SINMM = float(np.sin(np.pi - MARGIN) * MARGIN)
EPS = 1e-12
WS = 16.0                # fp8 pre-scale (per operand); PSUM = WS^2 * cos
NAUX = 2 * E + 1         # per-tile aux row: xw | wtar | redw
LB_SHIFT = float(40.0 * np.log(2.0))  # ln-range shift, re-added host-side
DOUBLE_ROW = True       # fp8 DoubleRow perf mode for the mains

_graph_cache = {}


def _build(nb, double_row=DOUBLE_ROW):
    """Build the per-core Bass graph for nb weight bands (nb % BPT == 0)."""
    from contextlib import ExitStack
    import concourse.bacc as bacc
    import concourse.tile as tile
    from concourse import mybir

    f32 = mybir.dt.float32
    bf16 = mybir.dt.bfloat16
    fp8 = mybir.dt.float8e4
    i32 = mybir.dt.int32
    A = mybir.AluOpType
    AF = mybir.ActivationFunctionType

    T = nb // BPT
    nc = bacc.Bacc(None)

    wt_ext = nc.declare_dram_parameter("wt", [nb, 128, KE, C], fp8, isOutput=False)
    xt_ext = nc.declare_dram_parameter("xt", [128, T, KE, 128], fp8, isOutput=False)
    aux_ext = nc.declare_dram_parameter("aux", [128, T, NAUX], bf16, isOutput=False)
    out_ext = nc.declare_dram_parameter("out", [128, T], f32, isOutput=True)

    with tile.TileContext(nc) as tc, ExitStack() as ctx:
        wpool = ctx.enter_context(tc.tile_pool(name="w", bufs=nb))
        cpool = ctx.enter_context(tc.tile_pool(name="const", bufs=1))
        vpool = ctx.enter_context(tc.tile_pool(name="vec", bufs=2))
        pmain = ctx.enter_context(tc.tile_pool(name="pmain", bufs=8, space="PSUM"))

        # DMA triggers: bands alternate sync/scalar HW-DGE rings so
        # descriptor generation is parallel and bands arrive in index order;
        # xt first on scalar (mains need it), aux early for the margin
        # pre-compute.
        w_tiles = []
        for b in range(nb):
            wt = wpool.tile([128, KE, C], fp8, tag="wt", name=f"wt{b}")
            w_tiles.append(wt)
        xt_sb = cpool.tile([128, T, KE, 128], fp8, tag="xt")
        aux_sb = cpool.tile([128, T, NAUX], bf16, tag="aux")

        # preload the natural_log_exp_and_others ACT table set (exp, ln):
        # one resident set => zero mid-kernel table loads. Before the scalar
        # ring's DMA triggers (they share the ACT compute queue).
        nc.scalar.add_instruction(mybir.InstLoadActFuncSet(
            name="preload-actset-6", act_func_set_id=6, ins=[], outs=[]))

        # Everything rides the sync ring: its queue has nothing else, so
        # ring-depth stalls on later triggers block nothing, and the small
        # xt/aux inputs aren't starved behind the band stream by engine
        # arbitration (the scalar ring gets poor service while the sync ring
        # hammers all 16 DMA engines). The ACT compute queue stays free for
        # the exps.
        nc.sync.dma_start(out=xt_sb[:], in_=xt_ext[:])
        nc.sync.dma_start(out=aux_sb[:], in_=aux_ext[:])
        for b in range(nb):
            nc.sync.dma_start(out=w_tiles[b][:], in_=wt_ext[b])

        # margin pre-compute, both tiles batched as [128, T] columns:
        # t = <xn, wn_target>; ft = t>theta ? t*cos_m - sqrt(1-t^2)*sin_m
        #                                  : t - sinmm   (labels always valid)
        tcos = cpool.tile([128, T], f32, tag="tcos")
        for t in range(T):
            tscr = vpool.tile([128, E], f32, tag="tscr")
            nc.vector.tensor_tensor(tscr[:], aux_sb[:, t, 0:E],
                                    aux_sb[:, t, E:2 * E], A.mult)
            nc.vector.reduce_sum(tcos[:, t:t + 1], tscr[:], axis=mybir.AxisListType.X)
        t2 = vpool.tile([128, T], f32, tag="t2")
        nc.vector.tensor_tensor(t2[:], tcos[:], tcos[:], A.mult)
        nc.vector.tensor_scalar(t2[:], t2[:], -1.0, 1.0, op0=A.mult, op1=A.add)
        nc.vector.tensor_scalar_max(t2[:], t2[:], 0.0)
        # sin_t = z*rsqrt(z): Quake seed + 2 Newton iterations on DVE
        yrs = vpool.tile([128, T], f32, tag="yrs")
        yi = yrs.bitcast(i32)
        nc.vector.tensor_scalar(yi[:], t2.bitcast(i32)[:], 1, None, op0=A.arith_shift_right)
        nc.vector.tensor_scalar(yi[:], yi[:], -1, 0x5F3759DF, op0=A.mult, op1=A.add)
        hz = vpool.tile([128, T], f32, tag="hz")
        nc.vector.tensor_scalar_mul(hz[:], t2[:], 0.5)
        y2 = vpool.tile([128, T], f32, tag="y2")
        for _ in range(2):
            nc.vector.tensor_tensor(y2[:], yrs[:], yrs[:], A.mult)
            nc.vector.tensor_tensor(y2[:], y2[:], hz[:], A.mult)
            nc.vector.tensor_scalar(y2[:], y2[:], -1.0, 1.5, op0=A.mult, op1=A.add)
            nc.vector.tensor_tensor(yrs[:], yrs[:], y2[:], A.mult)
        sint = vpool.tile([128, T], f32, tag="sint")
        nc.vector.tensor_tensor(sint[:], t2[:], yrs[:], A.mult)
        ctm = vpool.tile([128, T], f32, tag="ctm")
        nc.vector.tensor_scalar_mul(ctm[:], tcos[:], COS_M)
        sinm = vpool.tile([128, T], f32, tag="sinm")
        nc.vector.tensor_scalar_mul(sinm[:], sint[:], SIN_M)
        nc.vector.tensor_tensor(ctm[:], ctm[:], sinm[:], A.subtract)
        tms = vpool.tile([128, T], f32, tag="tms")
        nc.vector.tensor_scalar_add(tms[:], tcos[:], -SINMM)
        gt = vpool.tile([128, T], i32, tag="gt")
        nc.vector.tensor_scalar(gt[:], tcos[:], THETA, None, op0=A.is_gt)
        ft = vpool.tile([128, T], f32, tag="ft")
        nc.vector.select(ft[:], gt[:], ctm[:], tms[:])
        tf64 = cpool.tile([128, 3 * T], f32, tag="tf64")
        nc.vector.tensor_scalar_mul(tf64[:, 0:T], tcos[:], SCALE)
        nc.vector.tensor_scalar_mul(tf64[:, T:2 * T], ft[:], SCALE)
        # -64ft - LB_SHIFT: the shift keeps ln's argument inside the scalar
        # engine's +-2^64 range; the host adds LB_SHIFT back per sample
        nc.vector.tensor_scalar(tf64[:, 2 * T:3 * T], ft[:], -SCALE, -LB_SHIFT,
                                op0=A.mult, op1=A.add)
        # eb = exp(tf64) is emitted lazily in the first tail so it sits on
        # the ACT queue AFTER tile0's chunk exps (it's only needed at fold
        # time, and emitting it early would chain the exps behind the margin
        # pre-compute).
        # per tile t: exp(64t)=eb[:,t], exp(64ft)=eb[:,T+t], exp(-64ft)=eb[:,2T+t]
        # ebd4 = (exp(64ft) - exp(64t)) / NCC folds the target-logit swap into
        # the per-chunk-sum reduction bias
        eb = cpool.tile([128, 3 * T], f32, tag="eb")
        ebd4 = cpool.tile([128, T], f32, tag="ebd4")
        eb_emitted = []

        def emit_eb():
            if eb_emitted:
                return
            eb_emitted.append(True)
            nc.scalar.activation(eb[:], tf64[:], AF.Exp)
            nc.vector.tensor_tensor(ebd4[:], eb[:, T:2 * T], eb[:, 0:T], A.subtract)
            nc.vector.tensor_scalar_mul(ebd4[:], ebd4[:], 1.0 / NCC)

        escale = SCALE / (WS * WS)   # exp(escale * psum) = exp(64*cos)

        cps_t = {}
        ses_t = {}

        def mm(t, cc, k, j, cps):
            if double_row:
                if k != 0:
                    return
                nc.tensor.matmul(
                    cps[cc][NG * j:NG * (j + 1), :],
                    xt_sb[:, t, 0:KE, NG * j: NG * (j + 1)],
                    w_tiles[BPT * t + j][:, 0:KE, 512 * cc: 512 * cc + 512],
                    start=True, stop=True,
                    perf_mode=mybir.MatmulPerfMode.DoubleRow,
                    tile_position=(0, NG * j),
                )
            else:
                nc.tensor.matmul(
                    cps[cc][NG * j:NG * (j + 1), :],
                    xt_sb[:, t, k, NG * j: NG * (j + 1)],
                    w_tiles[BPT * t + j][:, k, 512 * cc: 512 * cc + 512],
                    start=(k == 0), stop=(k == KE - 1),
                    tile_position=(0, NG * j),
                )

        def emit_mains(t, defer_last):
            """Matmul order (cc, k, j): j innermost so the 4 bands' matmuls
            run concurrently on distinct PE column quadrants, cc outermost so
            PSUM chunks complete (and exp) one at a time. For the last tile
            the final band (latest DMA) is deferred per-chunk so the in-order
            PE queue drains all other work before waiting on it, and each
            chunk still completes (and exps) as early as possible."""
            cps_t[t] = [pmain.tile([128, 512], f32, tag="cos", name=f"cos{t}_{cc}")
                        for cc in range(NCC)]
            cps = cps_t[t]
            js = range(BPT - 1) if defer_last else range(BPT)
            for cc in range(NCC):
                for j in js:
                    for k in range(KE):
                        mm(t, cc, k, j, cps)
            if defer_last:
                for cc in range(NCC):
                    for k in range(KE):
                        mm(t, cc, k, BPT - 1, cps)
                    emit_exps(t, [cc])

        lb_sb = cpool.tile([128, T], f32, tag="lb")

        def emit_exps(t, ccs):
            """exp over PSUM chunk(s) with fused class-axis accumulation"""
            if t not in ses_t:
                ses_t[t] = cpool.tile([128, NCC], f32, tag=f"ses{t}",
                                      name=f"ses{t}")
            cps = cps_t[t]
            for cc in ccs:
                escr = vpool.tile([128, 512], bf16, tag="escr")
                nc.scalar.activation(escr[:], cps[cc][:], AF.Exp, scale=escale,
                                     accum_out=ses_t[t][:, cc:cc + 1])

        def emit_tail(t):
            """CE epilogue, entirely on ACT:
            se2 = sum(ses) + NCC*ebd4 = sumexp - exp(64t) + exp(64ft)
            lb  = ln(se2 * exp(-64ft)) = ln(se2) - 64 ft"""
            emit_eb()
            sescr = vpool.tile([128, NCC], f32, tag="sescr")
            se2 = cpool.tile([128, 1], f32, tag=f"se2_{t}")
            nc.scalar.activation(sescr[:], ses_t[t][:], AF.Identity,
                                 bias=ebd4[:, t:t + 1], accum_out=se2[:])
            nc.scalar.activation(lb_sb[:, t:t + 1], se2[:], AF.Ln,
                                 scale=eb[:, 2 * T + t:2 * T + t + 1])

        for t in range(T):
            emit_mains(t, defer_last=(t == T - 1))
            if t < T - 1:
                emit_exps(t, range(NCC))
            emit_tail(t)

        nc.sync.dma_start(out=out_ext[:], in_=lb_sb[:])

    nc.compile()
    return nc


def _pack(logits, labels, weight):
    """Route samples to the core owning their group; build per-core inputs."""
    logits = np.asarray(logits, dtype=np.float32)
    labels = np.asarray(labels).astype(np.int64)
    weight = np.asarray(weight, dtype=np.float32)

    group = (labels // C).astype(np.int64)
    local = (labels % C).astype(np.int64)
    core = group // GPC
    gl = group % GPC

    # host-side l2 normalization + fp8 quantization (x16 keeps the values
    # in fp8e4's normal range; cos is invariant to the row scaling)
    xn = logits / np.maximum(
        np.sqrt(np.sum(logits * logits, axis=1, keepdims=True)), EPS)
    wn2 = np.sqrt(np.einsum("gce,gce->gc", weight, weight))[:, :, None]
    wn = weight / np.maximum(wn2, EPS)
    wq = (WS * wn).astype(FP8)                    # (G, C, E) fp8 table
    xq = (WS * xn).astype(FP8)                    # (B, E)
    xw_all = (xq.astype(np.float32) / WS).astype(BF16)
    wtar_all = (wq[group, local].astype(np.float32) / WS).astype(BF16)

    # band assignment: per (core, local-group), ceil(count/NG) bands
    percg = [[np.nonzero((core == c) & (gl == g))[0] for g in range(GPC)]
             for c in range(NCORES)]
    nbands = [sum(max(1, -(-len(idx) // NG)) for idx in percg[c])
              for c in range(NCORES)]
    nb = max(nbands)
    nb = -(-nb // BPT) * BPT  # round up to full sample tiles
    T = nb // BPT

    in_maps = []
    valid_rows = []
    for c in range(NCORES):
        # band -> (group, sample indices)
        bands = []
        for g in range(GPC):
            idx = percg[c][g]
            nslice = max(1, -(-len(idx) // NG))
            for s in range(nslice):
                bands.append((g, idx[s * NG:(s + 1) * NG]))
        while len(bands) < nb:
            bands.append((0, np.empty(0, dtype=np.int64)))

        wt = np.empty((nb, 128, KE, C), dtype=FP8)
        xqp = np.zeros((T, 128, E), dtype=FP8)
        aux = np.zeros((128, T, NAUX), dtype=BF16)
        valid = np.zeros((128, T), dtype=bool)
        for b, (g, idx) in enumerate(bands):
            wg = wq[c * GPC + g]                     # (C, E) fp8
            for k in range(KE):
                wt[b, :, k, :] = wg[:, k * 128:(k + 1) * 128].T
            t, j = b // BPT, b % BPT
            sl = slice(NG * j, NG * j + len(idx))
            xqp[t, sl, :] = xq[idx]
            aux[sl, t, 0:E] = xw_all[idx]
            aux[sl, t, E:2 * E] = wtar_all[idx]
            aux[sl, t, 2 * E] = BF16(1.0 / B)
            valid[sl, t] = True
        # xt[p, t, k, r] = xq[t][r, k*128+p]
        xt = np.ascontiguousarray(
            np.transpose(xqp.reshape(T, 128, KE, 128), (3, 0, 2, 1)))
        in_maps.append({"wt": wt, "xt": xt, "aux": aux})
        valid_rows.append(valid)
    return in_maps, nb, valid_rows


def _run(logits, labels, weight, trace=False, **kw):
    from concourse.bass_utils import run_bass_kernel_spmd

    in_maps, nb, valid_rows = _pack(logits, labels, weight)
    nc = _graph_cache.get(nb)
    if nc is None:
        nc = _build(nb)
        _graph_cache[nb] = nc
    res = run_bass_kernel_spmd(nc, in_maps, core_ids=list(range(NCORES)),
                               trace=trace, **kw)
    total = sum(
        float(np.asarray(res.results[i]["out"], dtype=np.float32)[valid_rows[i]].sum())
        for i in range(NCORES)) / B + LB_SHIFT
    return np.asarray(total, dtype=np.float32), res


def kernel(logits, labels, weight):
    loss, _ = _run(logits, labels, weight)
    return loss
